# revision 1
# baseline (speedup 1.0000x reference)
"""Trainium2 Bass kernel for nn_CLIP_Embedding_35613868818658.

CNN stem (3x conv1d+GroupNorm+ReLU, 768->128->256->512) -> LayerNorm ->
bidirectional Mamba (selective scan, d_inner=1024, d_state=16, L=1024) ->
out_proj + residual.  Output (2, 512, 1024) f32.

Sharding: 2 batch-groups x 4-way d_inner split (DSH=256 rows per core).
Cores 0-3 handle b=0, cores 4-7 handle b=1; core g within a group owns
d_inner rows [256g, 256(g+1)).  Two in-group AllReduces: x_dbl (B/C/dt
projections, contracted over sharded d_inner) and the final out_proj.

The selective scan runs as 16 (one per state index s) hardware
tensor_tensor_scan instructions per d-tile over a [128, 2048] layout that
concatenates the forward and (time-reversed) backward directions along the
free axis; a[, t=0|1024] = 0 resets the recurrence at segment starts.
"""

import numpy as np
import ml_dtypes

import concourse.bass as bass
import concourse.mybir as mybir
import concourse.tile as tile
from contextlib import ExitStack

BF16 = ml_dtypes.bfloat16
F32 = mybir.dt.float32
BF = mybir.dt.bfloat16

B, CIN, L = 2, 768, 1024
DM, DI, DS, DTR, DC = 512, 1024, 16, 32, 4
NCORES, NGRP = 8, 4
DSH = DI // NGRP          # 256 d_inner rows per core
NDT = DSH // 128          # 2 d-tiles of 128 partitions
T2 = 2 * L                # fwd|rev concatenated time axis
EPS = 1e-5

AluOp = mybir.AluOpType
ActFn = mybir.ActivationFunctionType


def _ap_bcast_dram(handle, offset, dims):
    """Raw AP on a DRAM tensor: dims is a list of [step, count]."""
    return bass.AP(tensor=handle, offset=offset, ap=[list(d) for d in dims])


def split_excess_waits(nc, max_waits=1):
    """Walrus rejects instructions carrying more sync waits than the ISA
    encoding has slots for (1 on this toolchain).  Move excess waits onto
    preceding same-engine NoOps."""
    for bb in nc.main_func.blocks:
        insts = bb.instructions
        out, changed = [], False
        for ins in insts:
            si = ins.sync_info
            if si is not None and si.on_wait is not None and len(si.on_wait) > max_waits:
                waits = list(si.on_wait)
                keep, rest = waits[:max_waits], waits[max_waits:]
                idx = 0
                while rest:
                    chunk, rest = rest[:max_waits], rest[max_waits:]
                    nop = mybir.InstNoOp(
                        name=f"{ins.name}-wsplit{idx}",
                        engine=ins.engine,
                        sync_info=mybir.SyncInfo(on_wait=chunk, on_update=[]),
                        bass_nofuse=True,
                    )
                    out.append(nop)
                    idx += 1
                ins.sync_info = mybir.SyncInfo(
                    on_wait=keep, on_update=list(si.on_update or [])
                )
                changed = True
            out.append(ins)
        if changed:
            bb.instructions = out


def build_program(a_vals, split_waits=True, debug_dumps=False, reps=1, upto='full'):
    """Build the SPMD Bass program.  a_vals: 16 negative floats, A[s] = -(s+1)
    (verified d-independent and equal for both directions on the host)."""
    nc = bass.Bass("TRN2", target_bir_lowering=False, debug=False,
                   num_devices=NCORES)

    dt_in = lambda n, s, d=BF: nc.dram_tensor(n, list(s), d, kind="ExternalInput")

    x_in = dt_in("x", (CIN, L + 2))                      # host-padded, bf16
    w1T = dt_in("w1T", (3, 6, 128, 128))
    w2T = dt_in("w2T", (3, 1, 128, 256))
    w3T = dt_in("w3T", (3, 2, 128, 512))
    cb1 = dt_in("cb1", (128, 1), F32)
    cb2 = dt_in("cb2", (256, 1), F32)
    cb3 = dt_in("cb3", (512, 1), F32)
    gng1 = dt_in("gng1", (128, 1), F32)
    gnb1 = dt_in("gnb1", (128, 1), F32)
    gng2 = dt_in("gng2", (256, 1), F32)
    gnb2 = dt_in("gnb2", (256, 1), F32)
    gng3 = dt_in("gng3", (512, 1), F32)
    gnb3 = dt_in("gnb3", (512, 1), F32)
    onehot = dt_in("onehot", (3, 128, 32))
    ones_col = dt_in("ones_col", (128, 1))
    inprojT = dt_in("inprojT", (4, 128, 512))
    augT = dt_in("augT", (2, 512))
    xpT = dt_in("xpT", (2, 2, 128, 64))                 # [dir][ktile]
    dtT = dt_in("dtT", (2, 32, 256))                    # [dir]
    ndtb = dt_in("ndtb", (2, 256, 1), F32)              # -dt_b
    cvw = dt_in("cvw", (2, 256, 4), F32)
    cvb = dt_in("cvb", (2, 256, 1), F32)
    Dcol = dt_in("Dcol", (2, 256, 1), F32)
    outT = dt_in("outT", (2, 128, 512))                 # [dtile]

    out_ext = nc.dram_tensor("out", [DM, L], F32, kind="ExternalOutput")

    with tile.TileContext(nc) as tc, ExitStack() as ctx:
        P = 128
        consts = ctx.enter_context(tc.tile_pool(name="consts", bufs=1))
        psum = ctx.enter_context(tc.tile_pool(name="psum", bufs=3, space="PSUM"))
        mid = ctx.enter_context(tc.tile_pool(name="mid", bufs=1))
        dram = ctx.enter_context(tc.tile_pool(name="dram", bufs=1, space="DRAM"))
        sync, vec, pool, act, pe = nc.sync, nc.vector, nc.gpsimd, nc.scalar, nc.tensor

        # ---------------- consts to SBUF ----------------
        def load(poolh, shape, src, dtype=BF, name=None):
            t = poolh.tile(list(shape), dtype, tag=name)
            sync.dma_start(t[:], src)
            return t

        w1 = [[load(consts, (P, 128), w1T[k, ct], name=f"w1_{k}_{ct}")
               for ct in range(6)] for k in range(3)]
        w2 = [[load(consts, (P, 256), w2T[k, ct], name=f"w2_{k}_{ct}")
               for ct in range(1)] for k in range(3)]
        w3 = [[load(consts, (P, 512), w3T[k, ct], name=f"w3_{k}_{ct}")
               for ct in range(2)] for k in range(3)]
        def load_cols(dramt, co, name, width=1):
            return [load(consts, (128, width), dramt[mt * 128:(mt + 1) * 128, :],
                         F32, f"{name}{mt}") for mt in range(co // 128)]

        cbs = [load_cols(cb1, 128, "cb1"), load_cols(cb2, 256, "cb2"),
               load_cols(cb3, 512, "cb3")]
        gngs = [load_cols(gng1, 128, "gng1"), load_cols(gng2, 256, "gng2"),
                load_cols(gng3, 512, "gng3")]
        gnbs = [load_cols(gnb1, 128, "gnb1"), load_cols(gnb2, 256, "gnb2"),
                load_cols(gnb3, 512, "gnb3")]
        oneh = [load(consts, (P, 32), onehot[i], name=f"onehot{i}")
                for i in range(3)]
        ones1 = load(consts, (P, 1), ones_col[:], name="ones1")
        ipT = [load(consts, (P, 512), inprojT[kt], name=f"ipT{kt}") for kt in range(4)]
        augTs = load(consts, (2, 512), augT[:], name="augT")
        xpTs = [[load(consts, (P, 64), xpT[d, kt], name=f"xpT{d}{kt}")
                 for kt in range(2)] for d in range(2)]
        dtTs = [load(consts, (32, 256), dtT[d], name=f"dtT{d}") for d in range(2)]
        ndtbs = [[load(consts, (128, 1), ndtb[d, dt * 128:(dt + 1) * 128, :], F32,
                       f"ndtb{d}{dt}") for dt in range(2)] for d in range(2)]
        cvws = [[load(consts, (128, 4), cvw[d, dt * 128:(dt + 1) * 128, :], F32,
                      f"cvw{d}{dt}") for dt in range(2)] for d in range(2)]
        cvbs = [[load(consts, (128, 1), cvb[d, dt * 128:(dt + 1) * 128, :], F32,
                      f"cvb{d}{dt}") for dt in range(2)] for d in range(2)]
        Dcols = [[load(consts, (128, 1), Dcol[d, dt * 128:(dt + 1) * 128, :], F32,
                       f"D{d}{dt}") for dt in range(2)] for d in range(2)]
        outTs = [load(consts, (P, 512), outT[dt], name=f"outT{dt}") for dt in range(2)]

        epsc = consts.tile([128, 1], F32, tag="epsc")
        vec.memset(epsc[:], EPS)

        # DRAM scratch
        gn_scr = dram.tile([32, 2], F32, tag="gn_scr")
        ln_scr = dram.tile([1, L], F32, tag="ln_scr")
        xdbl_loc = dram.tile([2, 64, L], F32, tag="xdbl_loc")
        xdbl_red = dram.tile([2, 64, L], F32, tag="xdbl_red")
        bmc = dram.tile([2, 32, L], BF, tag="bmc")
        out_loc = dram.tile([DM, L], F32, tag="out_loc")
        out_red = dram.tile([DM, L], F32, tag="out_red")

        for rep in range(reps):
            fctx = ExitStack()
            stem = fctx.enter_context(tc.tile_pool(name=f"stem{rep}", bufs=1))
            stemtmp = fctx.enter_context(tc.tile_pool(name=f"stemtmp{rep}", bufs=3))
            statp = fctx.enter_context(tc.tile_pool(name=f"statp{rep}", bufs=2))
            rows = fctx.enter_context(tc.tile_pool(name=f"rows{rep}", bufs=1))
            x_t = [load(stem, (P, L + 2), x_in[ct * P:(ct + 1) * P, :],
                        name=f"x{ct}") for ct in range(6)]
            # ---------------- CNN stem ----------------
            def conv_gn_relu(layer, in_tiles, ws, cb, gng, gnb, co, out_f32):
                """in_tiles: list of padded (128, L+2) bf16; returns list of
                normalized+relu'd output tiles.  out_f32: emit f32 (for res)."""
                n_ct = len(in_tiles)
                n_co = co // 128
                cg = co // 32            # channels per group
                ngt = 128 // cg          # groups per 128-channel tile
                group_elems = float(cg) * L
                outs = []
                for mt in range(n_co):
                    h_raw = stemtmp.tile([P, L], F32, tag="h_raw")
                    stat4 = statp.tile([P, 4], F32, tag="stat4")
                    sq = stemtmp.tile([P, 512], BF, tag="sq")
                    for n in range(2):
                        ps = psum.tile([P, 512], F32, tag="ps_main", name="ps")
                        nmm = n_ct * 3
                        i = 0
                        for ct in range(n_ct):
                            for k in range(3):
                                pe.matmul(
                                    ps[:],
                                    ws[k][ct][:, mt * 128:(mt + 1) * 128],
                                    in_tiles[ct][:, n * 512 + k: n * 512 + k + 512],
                                    start=(i == 0), stop=(i == nmm - 1),
                                )
                                i += 1
                        act.activation(h_raw[:, n * 512:(n + 1) * 512], ps[:],
                                       ActFn.Identity, bias=cb[mt][:],
                                       accum_out=stat4[:, n:n + 1])
                        act.activation(sq[:], h_raw[:, n * 512:(n + 1) * 512],
                                       ActFn.Square, accum_out=stat4[:, 2 + n:3 + n])
                    # group stats: per-partition sums -> per-group via one-hot matmul
                    stat4b = statp.tile([P, 4], BF, tag="stat4b")
                    vec.tensor_copy(stat4b[:], stat4[:])
                    gps = psum.tile([32, 4], F32, tag="ps_small", name="gps", bufs=2)
                    pe.matmul(gps[:], oneh[layer - 1][:], stat4b[:])
                    gsb = statp.tile([32, 4], F32, tag="gsb")
                    act.activation(gsb[:], gps[:], ActFn.Copy)
                    sx = statp.tile([32, 1], F32, tag="sx")
                    sq_g = statp.tile([32, 1], F32, tag="sq_g")
                    vec.tensor_add(sx[:], gsb[:, 0:1], gsb[:, 1:2])
                    vec.tensor_add(sq_g[:], gsb[:, 2:3], gsb[:, 3:4])
                    mean = statp.tile([32, 1], F32, tag="mean")
                    act.activation(mean[:], sx[:], ActFn.Copy, scale=1.0 / group_elems)
                    msq = statp.tile([32, 1], F32, tag="msq")
                    act.activation(msq[:], sx[:], ActFn.Square, scale=1.0 / group_elems)
                    var = statp.tile([32, 1], F32, tag="var")
                    vec.scalar_tensor_tensor(var[:], sq_g[:], 1.0 / group_elems, msq[:],
                                             AluOp.mult, AluOp.subtract)
                    sig_g = statp.tile([32, 1], F32, tag="sig_g")
                    act.activation(sig_g[:], var[:], ActFn.Sqrt, bias=epsc[:32, :])
                    rstd = statp.tile([32, 1], F32, tag="rstd")
                    vec.reciprocal(rstd[:], sig_g[:])
                    # pack [rstd, mean] and expand groups 32 -> channels 128
                    stat2 = statp.tile([32, 2], F32, tag="stat2")
                    vec.tensor_copy(stat2[:, 0:1], rstd[:])
                    vec.tensor_copy(stat2[:, 1:2], mean[:])
                    sync.dma_start(gn_scr[:], stat2[:])
                    ch2 = statp.tile([P, 2], F32, tag="ch2")
                    sync.dma_start(
                        ch2[:],
                        _ap_bcast_dram(gn_scr[:].tensor, gn_scr[:].offset,
                                       [[2, ngt], [0, cg], [1, 2]]),
                    )
                    scale_c = statp.tile([P, 1], F32, tag="scale_c")
                    vec.tensor_mul(scale_c[:], ch2[:, 0:1], gng[mt][:])
                    nmean_s = statp.tile([P, 1], F32, tag="nmean_s")
                    vec.tensor_mul(nmean_s[:], ch2[:, 1:2], scale_c[:])
                    bias_c = statp.tile([P, 1], F32, tag="bias_c")
                    vec.tensor_sub(bias_c[:], gnb[mt][:], nmean_s[:])
                    if out_f32:
                        h_out = mid.tile([P, L], F32, tag=f"res{mt}")
                        act.activation(h_out[:], h_raw[:], ActFn.Relu,
                                       scale=scale_c[:], bias=bias_c[:])
                    else:
                        h_out = stem.tile([P, L + 2], BF, tag=f"h{layer}_{mt}")
                        vec.memset(h_out[:, 0:1], 0.0)
                        vec.memset(h_out[:, L + 1:L + 2], 0.0)
                        act.activation(h_out[:, 1:L + 1], h_raw[:], ActFn.Relu,
                                       scale=scale_c[:], bias=bias_c[:])
                    outs.append(h_out)
                return outs

            h1 = conv_gn_relu(1, x_t, w1, cbs[0], gngs[0], gnbs[0], 128, False)
            h2 = conv_gn_relu(2, h1, w2, cbs[1], gngs[1], gnbs[1], 256, False)
            res = conv_gn_relu(3, h2, w3, cbs[2], gngs[2], gnbs[2], 512, True)

            h3b = []
            for mt in range(4):
                t = stem.tile([P, L], BF, tag=f"h3b{mt}")
                vec.tensor_copy(t[:], res[mt][:])
                h3b.append(t)

            if upto == 'stem':
                sync.dma_start(out_ext[0:128, :], res[0][:])
                fctx.close()
                continue
            # ---------------- LayerNorm stats (over channels, via matmuls) -------
            hsq = []
            for mt in range(4):
                t = stemtmp.tile([P, L], BF, tag="hsq")
                act.activation(t[:], h3b[mt][:], ActFn.Square)
                hsq.append(t)
            musum = rows.tile([1, L], F32, tag="musum")
            sqsum = rows.tile([1, L], F32, tag="sqsum")
            for n in range(2):
                mu_ps = psum.tile([1, 512], F32, tag="ps_row", name="mu_ps", bufs=2)
                for kt in range(4):
                    pe.matmul(mu_ps[:], ones1[:],
                              h3b[kt][:, n * 512:(n + 1) * 512],
                              start=(kt == 0), stop=(kt == 3))
                act.activation(musum[:, n * 512:(n + 1) * 512], mu_ps[:], ActFn.Copy)
                sq_ps = psum.tile([1, 512], F32, tag="ps_row", name="sq_ps", bufs=2)
                for kt in range(4):
                    pe.matmul(sq_ps[:], ones1[:],
                              hsq[kt][:, n * 512:(n + 1) * 512],
                              start=(kt == 0), stop=(kt == 3))
                act.activation(sqsum[:, n * 512:(n + 1) * 512], sq_ps[:], ActFn.Copy)
            nmu = rows.tile([1, L], F32, tag="nmu")
            vec.tensor_scalar_mul(nmu[:], musum[:], -1.0 / DM)
            msql = rows.tile([1, L], F32, tag="msql")
            act.activation(msql[:], musum[:], ActFn.Square, scale=1.0 / DM)
            varl = rows.tile([1, L], F32, tag="varl")
            vec.scalar_tensor_tensor(varl[:], sqsum[:], 1.0 / DM, msql[:],
                                     AluOp.mult, AluOp.subtract)
            sigma = rows.tile([1, L], F32, tag="sigma")
            act.activation(sigma[:], varl[:], ActFn.Sqrt, bias=epsc[:1, :])
            recip = rows.tile([1, L], F32, tag="recip")
            vec.reciprocal(recip[:], sigma[:])
            nmu_b = rows.tile([1, L], BF, tag="nmu_b")
            vec.tensor_copy(nmu_b[:], nmu[:])
            sig_b = rows.tile([1, L], BF, tag="sig_b")
            vec.tensor_copy(sig_b[:], sigma[:])
            aug = rows.tile([2, L], BF, tag="aug")
            sync.dma_start(aug[0:1, :], nmu_b[:])
            sync.dma_start(aug[1:2, :], sig_b[:])
            sync.dma_start(ln_scr[:], recip[:])
            rbc = rows.tile([P, L], F32, tag="rbc")
            sync.dma_start(
                rbc[:],
                _ap_bcast_dram(ln_scr[:].tensor, ln_scr[:].offset, [[0, P], [1, L]]),
            )

            # ---------------- in_proj (LN folded in) ----------------
            # xpad[dt]: (128, L+6) bf16, 3 zero cols each side; z[dt]: (128, L)
            xpad = []
            zt = []
            for dt in range(NDT):
                xp_ = mid.tile([P, L + 6], BF, tag=f"xpad{dt}")
                vec.memset(xp_[:, 0:3], 0.0)
                vec.memset(xp_[:, L + 3:L + 6], 0.0)
                xpad.append(xp_)
                zt.append(mid.tile([P, L], BF, tag=f"z{dt}", name=f"z{dt}"))
            for m in range(4):
                for n in range(2):
                    ps = psum.tile([P, 512], F32, tag="ps_main", name="ps")
                    for kt in range(4):
                        pe.matmul(ps[:], ipT[kt][:, m * 128:(m + 1) * 128],
                                  h3b[kt][:, n * 512:(n + 1) * 512],
                                  start=(kt == 0), stop=False)
                    pe.matmul(ps[:], augTs[:, m * 128:(m + 1) * 128],
                              aug[:, n * 512:(n + 1) * 512], start=False, stop=True)
                    if m < 2:
                        dst = xpad[m][:, 3 + n * 512: 3 + (n + 1) * 512]
                    else:
                        dst = zt[m - 2][:, n * 512:(n + 1) * 512]
                    vec.tensor_mul(dst, ps[:], rbc[:, n * 512:(n + 1) * 512])

            if upto == 'inproj':
                sync.dma_start(out_ext[0:128, :], res[0][:])
                fctx.close()
                continue
            fctx.close()  # free stem/LN scratch address space for the scan phase
            sctx = ExitStack()
            scanp = sctx.enter_context(tc.tile_pool(name=f"scanp{rep}", bufs=2))
            onep = sctx.enter_context(tc.tile_pool(name=f"onep{rep}", bufs=1))

            # ---------------- depthwise causal conv + silu ----------------
            u_cat = [mid.tile([P, T2], BF, tag=f"u{dt}", name=f"u{dt}") for dt in range(NDT)]
            for dt in range(NDT):
                for d in range(2):  # 0 = fwd, 1 = rev (tau domain)
                    wcol = cvws[d][dt][:]
                    bcol = cvbs[d][dt][:]
                    acc = [scanp.tile([P, L], BF, tag=f"dwacc{i}", name=f"dwacc{i}") for i in range(2)]
                    X = xpad[dt]
                    def xsl(k):
                        if d == 0:
                            return X[:, k:k + L]
                        return X[:, L + 5 - k:5 - k if 5 - k >= 0 else None:-1]
                    vec.tensor_scalar(acc[0][:], xsl(0), wcol[:, 0:1], bcol,
                                      AluOp.mult, AluOp.add)
                    for k in (1, 2, 3):
                        vec.scalar_tensor_tensor(acc[k % 2][:], xsl(k), wcol[:, k:k + 1],
                                                 acc[(k + 1) % 2][:],
                                                 AluOp.mult, AluOp.add)
                    sg = scanp.tile([P, L], BF, tag="dwsg")
                    act.activation(sg[:], acc[1][:], ActFn.Sigmoid)
                    vec.tensor_mul(u_cat[dt][:, d * L:(d + 1) * L], acc[1][:], sg[:])

            if upto == 'dw':
                sync.dma_start(out_ext[0:128, :], res[0][:])
                sctx.close()
                continue
            # ---------------- x_dbl projection + AllReduce ----------------
            for d in range(2):
                xsb = onep.tile([64, L], F32, tag="xsb")
                for n in range(2):
                    xps = psum.tile([64, 512], F32, tag="ps_main", name="xps")
                    for dt in range(NDT):
                        pe.matmul(xps[:], xpTs[d][dt][:],
                                  u_cat[dt][:, d * L + n * 512: d * L + (n + 1) * 512],
                                  start=(dt == 0), stop=(dt == 1))
                    act.activation(xsb[:, n * 512:(n + 1) * 512], xps[:], ActFn.Copy)
                sync.dma_start(xdbl_loc[d], xsb[:])
            pool.collective_compute(
                "AllReduce", AluOp.add,
                replica_groups=[[0, 1, 2, 3], [4, 5, 6, 7]],
                ins=[xdbl_loc[:].opt()],
                outs=[xdbl_red[:].opt()],
            )

            if upto == 'xdbl':
                sync.dma_start(out_ext[0:128, :], res[0][:])
                sctx.close()
                continue
            # ---------------- dt_proj -> m = -softplus = ln(sigmoid(-x)) --------
            m_cat = [mid.tile([P, T2], BF, tag=f"m{dt}", name=f"m{dt}") for dt in range(NDT)]
            for d in range(2):
                dtf = onep.tile([32, L], F32, tag="dtf")
                sync.dma_start(dtf[:], xdbl_red[d, 0:32, :])
                dtfb = scanp.tile([32, L], BF, tag="dtfb")
                vec.tensor_copy(dtfb[:], dtf[:])
                for dt in range(NDT):
                    for n in range(2):
                        ps = psum.tile([P, 512], F32, tag="ps_main", name="ps")
                        pe.matmul(ps[:], dtTs[d][:, dt * 128:(dt + 1) * 128],
                                  dtfb[:, n * 512:(n + 1) * 512])
                        sgm = scanp.tile([P, 512], F32, tag="sgm")
                        act.activation(sgm[:], ps[:], ActFn.Sigmoid, scale=-1.0,
                                       bias=ndtbs[d][dt][:])
                        act.activation(m_cat[dt][:, d * L + n * 512: d * L + (n + 1) * 512],
                                       sgm[:], ActFn.Ln)
                # Bm (negated) and Cm rows -> bf16 DRAM for per-s broadcasts
                bmr = onep.tile([32, L], F32, tag="bmr")
                sync.dma_start(bmr[:], xdbl_red[d, 32:64, :])
                bmcb = scanp.tile([32, L], BF, tag="bmcb")
                vec.tensor_copy(bmcb[:], bmr[:])
                sync.dma_start(bmc[d], bmcb[:])

            # mx = m * u = -(delta * u)
            mx = [mid.tile([P, T2], BF, tag=f"mx{dt}", name=f"mx{dt}") for dt in range(NDT)]
            for dt in range(NDT):
                vec.tensor_mul(mx[dt][:], m_cat[dt][:], u_cat[dt][:])

            if upto == 'dt':
                sync.dma_start(out_ext[0:128, :], res[0][:])
                sctx.close()
                continue
            # ---------------- selective scan ----------------
            bmc_ap = bmc[:]

            y_dt = []
            for dt in range(NDT):
                pending = {}
                for s in range(16):
                    a_s = scanp.tile([P, T2], BF, tag="a_s")
                    # a = exp(A_s * delta) = exp((-A_s) * m)
                    act.activation(a_s[:], m_cat[dt][:], ActFn.Exp,
                                   scale=float(-a_vals[s]))
                    vec.memset(a_s[:, 0:1], 0.0)
                    vec.memset(a_s[:, L:L + 1], 0.0)
                    Bs = scanp.tile([P, T2], BF, tag="Bs")
                    sync.dma_start(
                        Bs[:],
                        _ap_bcast_dram(bmc_ap.tensor, bmc_ap.offset + s * L,
                                       [[0, P], [32 * L, 2], [1, L]]),
                    )
                    Cs = scanp.tile([P, T2], BF, tag="Cs")
                    sync.dma_start(
                        Cs[:],
                        _ap_bcast_dram(bmc_ap.tensor, bmc_ap.offset + (16 + s) * L,
                                       [[0, P], [32 * L, 2], [1, L]]),
                    )
                    b_s = scanp.tile([P, T2], BF, tag="b_s")
                    vec.scalar_tensor_tensor(b_s[:], mx[dt][:], -1.0, Bs[:],
                                             AluOp.mult, AluOp.mult)
                    h_s = scanp.tile([P, T2], BF, tag="h_s")
                    vec.tensor_tensor_scan(h_s[:], a_s[:], b_s[:], 0.0,
                                           AluOp.mult, AluOp.add)
                    gs = scanp.tile([P, T2], BF, tag="gs")
                    pool.tensor_mul(gs[:], h_s[:], Cs[:])
                    # pairwise reduction tree over s (16 leaves -> 1 root)
                    node, level = gs, 0
                    while level in pending:
                        prev = pending.pop(level)
                        nxt = scanp.tile([P, T2], BF, tag=f"yacc{level + 1}", name="nxt",
                                         bufs=2)
                        pool.tensor_add(nxt[:], prev[:], node[:])
                        node, level = nxt, level + 1
                    pending[level] = node
                assert list(pending) == [4]
                y_dt.append(pending[4])

            if upto == 'scan':
                sync.dma_start(out_ext[0:128, :], res[0][:])
                sctx.close()
                continue
            # ---------------- combine directions, D-term, gate ----------------
            ygate = []
            for dt in range(NDT):
                ysum = onep.tile([P, L], BF, tag="ysum")
                vec.tensor_add(ysum[:], y_dt[dt][:, 0:L], y_dt[dt][:, T2 - 1:L - 1:-1])
                t1 = onep.tile([P, L], BF, tag="t1")
                vec.scalar_tensor_tensor(t1[:], u_cat[dt][:, 0:L],
                                         Dcols[0][dt][:], ysum[:],
                                         AluOp.mult, AluOp.add)
                t2 = onep.tile([P, L], BF, tag="ysum", name="t2")
                vec.scalar_tensor_tensor(t2[:], u_cat[dt][:, T2 - 1:L - 1:-1],
                                         Dcols[1][dt][:], t1[:],
                                         AluOp.mult, AluOp.add)
                sgz = onep.tile([P, L], BF, tag="sgz")
                act.activation(sgz[:], zt[dt][:], ActFn.Sigmoid)
                zs = onep.tile([P, L], BF, tag="zs")
                vec.tensor_mul(zs[:], zt[dt][:], sgz[:])
                yg = scanp.tile([P, L], BF, tag="yg")
                vec.tensor_mul(yg[:], t2[:], zs[:])
                ygate.append(yg)

            # ---------------- out_proj + residual + AllReduce ----------------
            for m in range(4):
                osb = onep.tile([P, L], F32, tag="osb")
                for n in range(2):
                    ps = psum.tile([P, 512], F32, tag="ps_main", name="ps")
                    for dt in range(NDT):
                        pe.matmul(ps[:], outTs[dt][:, m * 128:(m + 1) * 128],
                                  ygate[dt][:, n * 512:(n + 1) * 512],
                                  start=(dt == 0), stop=(dt == 1))
                    vec.scalar_tensor_tensor(osb[:, n * 512:(n + 1) * 512],
                                             res[m][:, n * 512:(n + 1) * 512],
                                             1.0 / NGRP, ps[:],
                                             AluOp.mult, AluOp.add)
                sync.dma_start(out_loc[m * 128:(m + 1) * 128, :], osb[:])
            pool.collective_compute(
                "AllReduce", AluOp.add,
                replica_groups=[[0, 1, 2, 3], [4, 5, 6, 7]],
                ins=[out_loc[:].opt()],
                outs=[out_red[:].opt()],
            )
            sync.dma_start(out_ext[:], out_red[:])
            if debug_dumps and rep == reps - 1:
                def dump(name, tiles, shape, dt=BF):
                    dtile = dram.tile(list(shape), dt, tag=name, name=name)
                    if not isinstance(tiles, list):
                        tiles = [tiles]
                    for i, t in enumerate(tiles):
                        sync.dma_start(dtile[i], t[:])
                    return dtile
                dump("dbg_res", res, (4, P, L), F32)
                dump("dbg_xpad", xpad, (2, P, L + 6))
                dump("dbg_z", zt, (2, P, L))
                dump("dbg_u", u_cat, (2, P, T2))
                dump("dbg_m", m_cat, (2, P, T2))
                dump("dbg_mx", mx, (2, P, T2))
                dump("dbg_y", y_dt, (2, P, T2))
                dump("dbg_yg", ygate, (2, P, L))
            sctx.close()

    if split_waits:
        split_excess_waits(nc)
    return nc


def prep_inputs(inputs):
    """Host-side sharding/weight prep.  Returns (a_vals, in_maps)."""
    f32 = lambda a: np.ascontiguousarray(np.asarray(a, np.float32))
    bf = lambda a: np.ascontiguousarray(np.asarray(a, np.float32).astype(BF16))

    A_f = -np.exp(f32(inputs["Alog_f"]))
    A_r = -np.exp(f32(inputs["Alog_r"]))
    assert np.abs(A_f - A_f[0:1]).max() < 1e-5, "A not d-independent"
    assert np.abs(A_f - A_r).max() < 1e-5, "A_f != A_r"
    a_vals = [float(v) for v in A_f[0]]

    x = f32(inputs["x"])
    w1 = f32(inputs["conv1_w"]); w2 = f32(inputs["conv2_w"]); w3 = f32(inputs["conv3_w"])
    w1T = bf(np.transpose(w1, (2, 1, 0)).reshape(3, 6, 128, 128))
    w2T = bf(np.transpose(w2, (2, 1, 0)).reshape(3, 1, 128, 256))
    w3T = bf(np.transpose(w3, (2, 1, 0)).reshape(3, 2, 128, 512))
    onehot = np.zeros((3, 128, 32), np.float32)
    for i, cg in enumerate((4, 8, 16)):
        onehot[i, np.arange(128), np.arange(128) // cg] = 1.0
    ln_g = f32(inputs["ln_g"]); ln_b = f32(inputs["ln_b"])
    ipw = f32(inputs["in_proj_w"])
    opw = f32(inputs["out_proj_w"])

    common = dict(
        w1T=w1T, w2T=w2T, w3T=w3T,
        cb1=f32(inputs["conv1_b"]).reshape(128, 1),
        cb2=f32(inputs["conv2_b"]).reshape(256, 1),
        cb3=f32(inputs["conv3_b"]).reshape(512, 1),
        gng1=f32(inputs["gn1_g"]).reshape(128, 1),
        gnb1=f32(inputs["gn1_b"]).reshape(128, 1),
        gng2=f32(inputs["gn2_g"]).reshape(256, 1),
        gnb2=f32(inputs["gn2_b"]).reshape(256, 1),
        gng3=f32(inputs["gn3_g"]).reshape(512, 1),
        gnb3=f32(inputs["gn3_b"]).reshape(512, 1),
        onehot=bf(onehot),
        ones_col=bf(np.ones((128, 1), np.float32)),
    )

    in_maps = []
    for core in range(NCORES):
        b, grp = core // NGRP, core % NGRP
        rows = np.arange(grp * DSH, (grp + 1) * DSH)
        sel = np.concatenate([rows, DI + rows])
        Wsel = ipw[sel] * ln_g[None, :]
        inprojT = bf(Wsel.T.reshape(4, 128, 2 * DSH))
        augTm = bf(np.stack([Wsel.sum(1), ipw[sel] @ ln_b]))
        xpTm = np.stack([
            bf(f32(inputs[f"xp_w_{s}"])[:, rows].T.reshape(2, 128, 64))
            for s in ("f", "r")])
        dtTm = np.stack([
            bf(f32(inputs[f"dt_w_{s}"])[rows].T) for s in ("f", "r")])
        ndtbm = np.stack([
            -f32(inputs[f"dt_b_{s}"])[rows].reshape(DSH, 1) for s in ("f", "r")])
        cvwm = np.stack([
            f32(inputs[f"cv_w_{s}"])[rows, 0] for s in ("f", "r")])
        cvbm = np.stack([
            f32(inputs[f"cv_b_{s}"])[rows].reshape(DSH, 1) for s in ("f", "r")])
        Dm = np.stack([
            f32(inputs[f"D_{s}"])[rows].reshape(DSH, 1) for s in ("f", "r")])
        outTm = bf(opw[:, rows].T.reshape(2, 128, DM))
        xpadded = bf(np.pad(x[b], ((0, 0), (1, 1))))
        m = dict(common)
        m.update(x=xpadded, inprojT=inprojT, augT=augTm, xpT=xpTm, dtT=dtTm,
                 ndtb=ndtbm, cvw=cvwm, cvb=cvbm, Dcol=Dm, outT=outTm)
        in_maps.append(m)
    return a_vals, in_maps


def kernel(**inputs) -> np.ndarray:
    from concourse.bass_utils import run_bass_kernel_spmd
    a_vals, in_maps = prep_inputs(inputs)
    nc = build_program(a_vals)
    res = run_bass_kernel_spmd(nc, in_maps, list(range(NCORES)))
    out = np.stack([res.results[0]["out"], res.results[NGRP]["out"]])
    return np.ascontiguousarray(out.astype(np.float32))


if __name__ == "__main__":
    import reference as R
    import jax
    with jax.default_device(jax.devices("cpu")[0]):
        inp = {k: np.asarray(v) for k, v in R.setup_inputs().items()}
        ref = np.asarray(R.reference(**R.setup_inputs()))
    got = kernel(**inp)
    err = np.abs(got - ref).max() / np.abs(ref).max()
    print("Relative error:", err)

